# revision 14
# baseline (speedup 1.0000x reference)
"""Single-head causal attention (B=8, T=2048, D=512, H=64) on 8 TRN2 cores.

Data-parallel: one batch element per NeuronCore. Each core computes
attention in the S^T layout (keys on partitions, queries on the free axis):

  qT/kT/vT [64, T] = W.T @ x.T        (f32r matmuls, N=512 chunks)
  v        [T, 64] via PE transpose of vT, with a ones column appended
  S^T[j,i] = kT_jblock.T @ qT          (strips of causal width)
  P^T      = exp(S^T / 8)              (ScalarE, one op per strip;
                                        no max-subtraction: scores are
                                        bounded by ~|q||k|sqrt(H)/8 << 88)
  out^T[h,i], l[i] = [v|1]_jb.T @ P^T  (accumulated over j-blocks in PSUM;
                                        row 64 is the softmax denominator)

The kernel returns the unnormalized [65, T] strip per core; the host
divides by the denominator row and transposes back to [T, 64].
"""

import sys

sys.path.insert(0, "/opt/trn_rl_repo")

import numpy as np

import concourse.bass as bass
import concourse.mybir as mybir
import concourse.tile as tile

B, T, D, H = 8, 2048, 512, 64
N_CORES = 8
HALF = T // 2  # i-axis pass width

f32 = mybir.dt.float32
f32r = mybir.dt.float32r
f16 = mybir.dt.float16

_cache = {}


def _legalize_waits(nc, max_waits=1):
    """Walrus codegen accepts at most one sync wait per instruction; hoist
    extras onto same-engine NOPs placed immediately before (engine queues
    are FIFO so blocking semantics are unchanged)."""
    counter = 0
    for bb in nc.main_func.blocks:
        if not any(
            ins.sync_info is not None and len(ins.sync_info.on_wait) > max_waits
            for ins in bb.instructions
        ):
            continue
        new_list = []
        for ins in bb.instructions:
            si = ins.sync_info
            if si is not None and len(si.on_wait) > max_waits:
                waits = list(si.on_wait)
                hoist, keep = waits[:-max_waits], waits[-max_waits:]
                for w in hoist:
                    counter += 1
                    new_list.append(
                        mybir.InstNoOp(
                            name=f"I-waitfix-{counter}",
                            engine=ins.engine,
                            sync_info=mybir.SyncInfo(on_wait=[w], on_update=[]),
                            bass_nofuse=True,
                        )
                    )
                ins.sync_info = mybir.SyncInfo(
                    on_wait=keep, on_update=list(si.on_update)
                )
            new_list.append(ins)
        bb.instructions = new_list
    return counter


def _chunks(lo, hi, step, align):
    """Split [lo, hi) at multiples of `step` relative to `align`."""
    out = []
    cur = lo
    while cur < hi:
        nxt = min(hi, align + ((cur - align) // step + 1) * step)
        out.append((cur, nxt))
        cur = nxt
    return out


def _build():
    nc = bass.Bass()

    xt_d = nc.declare_dram_parameter("xt", [D, T], f32r, isOutput=False)
    # consts packed per partition: [wqk c0..c3 | wv c0..c3 | mask | ones | ident]
    CW = 4 * 128 + 4 * 64 + 128 + 16 + 64  # 976
    consts_d = nc.declare_dram_parameter("consts", [128, CW], f32r, isOutput=False)
    out_d = nc.declare_dram_parameter("out", [H + 1, T], f32, isOutput=True)

    NC_TILES = D // 128  # 4 c-tiles

    with tile.TileContext(nc) as tc:
        with (
            tc.tile_pool(name="const", bufs=1) as cpool,
            tc.tile_pool(name="xt", bufs=1) as xpool,
            tc.tile_pool(name="qkv", bufs=1) as qkvpool,
            tc.tile_pool(name="p", bufs=2) as ppool,
            tc.tile_pool(name="o", bufs=2) as opool,
            tc.tile_pool(name="ps_proj", bufs=1, space="PSUM") as ps_proj,
            tc.tile_pool(name="ps_vt", bufs=1, space="PSUM") as ps_vt,
            tc.tile_pool(name="ps_s", bufs=2, space="PSUM") as ps_s,
            tc.tile_pool(name="ps_pv", bufs=1, space="PSUM") as ps_pv,
        ):
            consts = cpool.tile([128, CW], f32r)
            nc.sync.dma_start(consts[:], consts_d[:])
            wqk = [consts[:, 128 * c : 128 * (c + 1)] for c in range(NC_TILES)]
            wv = [
                consts[:, 512 + 64 * c : 512 + 64 * (c + 1)] for c in range(NC_TILES)
            ]
            mask = consts[:, 768:896]
            ones = consts[:, 896:912]
            mask16 = cpool.tile([128, 128], f16)
            nc.vector.tensor_copy(mask16[:], mask)
            ident16 = cpool.tile([H, H], f16)
            nc.vector.tensor_copy(ident16[:], consts[0:H, 912:976])

            # HAM warmers: f32r matmuls don't register as PE activity for the
            # clock gate (like transpose-mode), leaving the array at 1.2 GHz.
            # Small bf16 matmuls sprinkled through the stream keep K=8/8.
            warm_bf = cpool.tile([128, 512], mybir.dt.bfloat16)
            nc.vector.memset(warm_bf[:], 1.0)
            warm_ps = ps_vt.tile([128, 512], f32, tag="warm")  # own bank

            def warmer(n=64):
                nc.tensor.matmul(
                    warm_ps[:, 0:n], warm_bf[:, 0:128], warm_bf[:, 0:n], start=True, stop=True
                )

            # dense burst covering one full HAM SHORT window right at the
            # start (PE is idle during input DMA anyway) to reach K=8/8.
            for _ in range(9):
                warmer(512)

            # host reorders xt so DRAM row (4p + c) holds x.T row (128c + p):
            # one DMA per half covers all four c-tiles with one 2D
            # descriptor per partition.
            xt_all = xpool.tile([128, NC_TILES, T], f32r)
            xt_src = xt_d.rearrange("(p c) t -> p c t", c=NC_TILES)
            xt = [xt_all[:, c, :] for c in range(NC_TILES)]
            qT = qkvpool.tile([H, T], f32r)
            kT = qkvpool.tile([H, T], f32r)
            vT = qkvpool.tile([H, T], f16)
            v1 = qkvpool.tile([128, T // 128, H + 1], f16)
            nc.vector.tensor_copy(v1[:, :, H : H + 1], ones)

            for h in range(2):
                t0 = h * HALF
                nc.sync.dma_start(
                    xt_all[:, :, t0 : t0 + HALF],
                    xt_src[:, :, t0 : t0 + HALF],
                )

                # --- projections for this half ---
                for tc512 in range(t0, t0 + HALF, 512):
                    warmer()
                    qk_ps = ps_proj.tile([128, 512], f32, tag="work")
                    for c in range(NC_TILES):
                        nc.tensor.matmul(
                            qk_ps[:],
                            wqk[c],
                            xt[c][:, tc512 : tc512 + 512],
                            start=(c == 0),
                            stop=(c == NC_TILES - 1),
                        )
                    nc.vector.tensor_copy(
                        qT[:, tc512 : tc512 + 512], qk_ps[0:H, :]
                    )
                    nc.vector.tensor_copy(
                        kT[:, tc512 : tc512 + 512], qk_ps[H : 2 * H, :]
                    )
                    v_ps = ps_proj.tile([128, 512], f32, tag="work")
                    for c in range(NC_TILES):
                        nc.tensor.matmul(
                            v_ps[0:H, :],
                            wv[c],
                            xt[c][:, tc512 : tc512 + 512],
                            start=(c == 0),
                            stop=(c == NC_TILES - 1),
                        )
                    nc.vector.tensor_copy(vT[:, tc512 : tc512 + 512], v_ps[0:H, :])

                # --- v transposes: vT [64, T] -> v1 [j, h] tiles ---
                vt_ps = ps_proj.tile([128, 8, H], f16, tag="work")
                for jl, jj in enumerate(range(8 * h, 8 * h + 8)):
                    nc.tensor.transpose(
                        vt_ps[:, jl, :], vT[:, 128 * jj : 128 * (jj + 1)], ident16[:]
                    )
                    nc.vector.tensor_copy(v1[:, jj, 0:H], vt_ps[:, jl, :])

                # --- attention pass over this half of i ---
                pv_ps = ps_pv.tile([H + 1, HALF], f32, tag="pv")
                n_jb = 8 * h + 8
                for jb in range(n_jb):
                    warmer()
                    i_start = max(t0, 128 * jb)
                    W = t0 + HALF - i_start
                    s_ps = ps_s.tile([128, HALF], f32, tag="s")
                    # S^T strip: chunk by strip-local 512 (PSUM bank) bounds
                    for ls, le in _chunks(0, W, 512, 0):
                        nc.tensor.matmul(
                            s_ps[:, ls:le],
                            kT[:, 128 * jb : 128 * (jb + 1)],
                            qT[:, i_start + ls : i_start + le],
                            start=True,
                            stop=True,
                        )
                        warmer()
                    p_sb = ppool.tile([128, HALF], f16, tag="p")
                    nc.scalar.activation(
                        p_sb[:, 0:W],
                        s_ps[:, 0:W],
                        mybir.ActivationFunctionType.Exp,
                        scale=1.0 / 8.0,
                    )
                    if 128 * jb >= t0:
                        nc.vector.tensor_mul(
                            p_sb[:, 0:128], p_sb[:, 0:128], mask16[:]
                        )
                    # PV accumulate: chunk by global-512 (pv bank) bounds
                    for gs, ge in _chunks(i_start, t0 + HALF, 512, 0):
                        ic_last_jb = min(n_jb - 1, (ge - 1) // 128)
                        nc.tensor.matmul(
                            pv_ps[:, gs - t0 : ge - t0],
                            v1[:, jb, :],
                            p_sb[:, gs - i_start : ge - i_start],
                            start=(jb == 0),
                            stop=(jb == ic_last_jb),
                        )
                out_sb = opool.tile([H + 1, HALF], f32, tag="o")
                nc.scalar.copy(out_sb[:], pv_ps[:])
                nc.sync.dma_start(out_d[:, t0 : t0 + HALF], out_sb[:])

    _legalize_waits(nc)
    return nc


def build_in_maps(x, Wq, Wk, Wv):
    x = np.ascontiguousarray(np.asarray(x), dtype=np.float32)
    wqk_np = np.ascontiguousarray(
        np.concatenate([np.asarray(Wq), np.asarray(Wk)], axis=1), dtype=np.float32
    )
    wv_np = np.ascontiguousarray(np.asarray(Wv), dtype=np.float32)
    # keep iff j_local <= i_local (upper-triangular in [j, i] layout)
    mask_np = np.triu(np.ones((128, 128), dtype=np.float32))
    ident_np = np.zeros((128, H), dtype=np.float32)
    ident_np[:H] = np.eye(H, dtype=np.float32)
    ones_np = np.ones((128, T // 128), dtype=np.float32)
    wqk_t = wqk_np.reshape(4, 128, 2 * H).transpose(1, 0, 2).reshape(128, 512)
    wv_t = wv_np.reshape(4, 128, H).transpose(1, 0, 2).reshape(128, 256)
    consts_np = np.ascontiguousarray(
        np.concatenate([wqk_t, wv_t, mask_np, ones_np, ident_np], axis=1)
    )
    return [
        {
            "xt": np.ascontiguousarray(
                x[b].T.reshape(4, 128, T).transpose(1, 0, 2).reshape(D, T)
            ),
            "consts": consts_np,
        }
        for b in range(N_CORES)
    ]


def kernel(x, Wq, Wk, Wv):
    from concourse.bass_utils import run_bass_kernel_spmd

    if "nc" not in _cache:
        _cache["nc"] = _build()
    nc = _cache["nc"]

    in_maps = build_in_maps(x, Wq, Wk, Wv)
    res = run_bass_kernel_spmd(nc, in_maps, list(range(N_CORES))).results

    out = np.empty((B, T, H), dtype=np.float32)
    for b in range(N_CORES):
        strip = res[b]["out"]  # [H+1, T]
        out[b] = (strip[:H, :] / strip[H : H + 1, :]).T
    return out


if __name__ == "__main__":
    rng = np.random.default_rng(0)
    x = rng.standard_normal((B, T, D)).astype(np.float32)
    s = 1.0 / np.sqrt(D)
    Wq = (rng.standard_normal((D, H)) * s).astype(np.float32)
    Wk = (rng.standard_normal((D, H)) * s).astype(np.float32)
    Wv = (rng.standard_normal((D, H)) * s).astype(np.float32)
    out = kernel(x=x, Wq=Wq, Wk=Wk, Wv=Wv)
    print("out", out.shape, out.dtype, np.abs(out).max())


# revision 30
# speedup vs baseline: 1.5638x; 1.5638x over previous
"""Single-head causal attention (B=8, T=2048, D=512, H=64) on 8 TRN2 cores.

Data-parallel: one batch element per NeuronCore. Each core computes
attention in the S^T layout (keys on partitions, queries on the free axis):

  qT/kT/vT [64, T] = W.T @ x.T        (f32r matmuls, N=512 chunks)
  v        [T, 64] via PE transpose of vT, with a ones column appended
  S^T[j,i] = kT_jblock.T @ qT          (strips of causal width)
  P^T      = exp(S^T / 8)              (ScalarE, one op per strip;
                                        no max-subtraction: scores are
                                        bounded by ~|q||k|sqrt(H)/8 << 88)
  out^T[h,i], l[i] = [v|1]_jb.T @ P^T  (accumulated over j-blocks in PSUM;
                                        row 64 is the softmax denominator)

The kernel returns the unnormalized [65, T] strip per core; the host
divides by the denominator row and transposes back to [T, 64].
"""

import sys

sys.path.insert(0, "/opt/trn_rl_repo")

import numpy as np

import concourse.bass as bass
import concourse.mybir as mybir
import concourse.tile as tile

B, T, D, H = 8, 2048, 512, 64
N_CORES = 8
HALF = T // 2  # i-axis pass width

f32 = mybir.dt.float32
f32r = mybir.dt.float32r
f16 = mybir.dt.float16

_cache = {}


def _legalize_waits(nc, max_waits=1):
    """Walrus codegen accepts at most one sync wait per instruction; hoist
    extras onto same-engine NOPs placed immediately before (engine queues
    are FIFO so blocking semantics are unchanged)."""
    counter = 0
    for bb in nc.main_func.blocks:
        if not any(
            ins.sync_info is not None and len(ins.sync_info.on_wait) > max_waits
            for ins in bb.instructions
        ):
            continue
        new_list = []
        for ins in bb.instructions:
            si = ins.sync_info
            if si is not None and len(si.on_wait) > max_waits:
                waits = list(si.on_wait)
                hoist, keep = waits[:-max_waits], waits[-max_waits:]
                for w in hoist:
                    counter += 1
                    new_list.append(
                        mybir.InstNoOp(
                            name=f"I-waitfix-{counter}",
                            engine=ins.engine,
                            sync_info=mybir.SyncInfo(on_wait=[w], on_update=[]),
                            bass_nofuse=True,
                        )
                    )
                ins.sync_info = mybir.SyncInfo(
                    on_wait=keep, on_update=list(si.on_update)
                )
            new_list.append(ins)
        bb.instructions = new_list
    return counter


def _chunks(lo, hi, step, align):
    """Split [lo, hi) at multiples of `step` relative to `align`."""
    out = []
    cur = lo
    while cur < hi:
        nxt = min(hi, align + ((cur - align) // step + 1) * step)
        out.append((cur, nxt))
        cur = nxt
    return out


def _build():
    nc = bass.Bass()

    xhi_d = nc.declare_dram_parameter("xhi", [D, T], f16, isOutput=False)
    # consts packed per partition (all fp16):
    # [wqk_hi c0..c3 | wqk_lo c0..c3 | wv c0..c3 | mask | ones | ident]
    CW = 512 + 512 + 256 + 128 + 16 + 64  # 1488
    consts_d = nc.declare_dram_parameter("consts", [128, CW], f16, isOutput=False)
    out_d = nc.declare_dram_parameter("out", [H + 1, T], f32, isOutput=True)

    NC_TILES = D // 128  # 4 c-tiles

    with tile.TileContext(nc) as tc:
        with (
            tc.tile_pool(name="const", bufs=1) as cpool,
            tc.tile_pool(name="xt", bufs=1) as xpool,
            tc.tile_pool(name="qkv", bufs=1) as qkvpool,
            tc.tile_pool(name="p", bufs=2) as ppool,
            tc.tile_pool(name="o", bufs=2) as opool,
            tc.tile_pool(name="ps_proj", bufs=2, space="PSUM") as ps_proj,
            tc.tile_pool(name="ps_s", bufs=2, space="PSUM") as ps_s,
            tc.tile_pool(name="ps_pv", bufs=1, space="PSUM") as ps_pv,
        ):
            consts = cpool.tile([128, CW], f16)
            nc.sync.dma_start(consts[:], consts_d[:])
            wqk_hi = [consts[:, 128 * c : 128 * (c + 1)] for c in range(NC_TILES)]
            wqk_lo = [
                consts[:, 512 + 128 * c : 512 + 128 * (c + 1)]
                for c in range(NC_TILES)
            ]
            wv = [
                consts[:, 1024 + 64 * c : 1024 + 64 * (c + 1)]
                for c in range(NC_TILES)
            ]
            mask16 = consts[:, 1280:1408]
            ones = consts[:, 1408:1424]
            ident16 = consts[0:H, 1424:1488]

            # initial HAM warm-up burst: one full SHORT window of dense bf16
            # matmuls while the input DMAs run, so the 2.4 GHz clock engages
            # before real work starts.
            warm_bf = cpool.tile([128, 512], mybir.dt.bfloat16)
            nc.vector.memset(warm_bf[:], 1.0)
            # touch Exp once so the ACT table set loads during the DMA phase
            exp_warm = cpool.tile([1, 2], f32)
            nc.scalar.activation(
                exp_warm[:], warm_bf[0:1, 0:2], mybir.ActivationFunctionType.Exp
            )
            warm_ps = ps_s.tile([128, 512], f32, tag="s", name="warm_ps")
            for _ in range(9):
                nc.tensor.matmul(
                    warm_ps[:], warm_bf[:, 0:128], warm_bf[:], start=True, stop=True
                )

            # host reorders x.T so DRAM row (4p + c) holds x.T row (128c + p):
            # one DMA per piece covers all four c-tiles with one 2D
            # descriptor per partition.
            xhi_all = xpool.tile([128, NC_TILES, T], f16)
            xhi_src = xhi_d.rearrange("(p c) t -> p c t", c=NC_TILES)
            xhi = [xhi_all[:, c, :] for c in range(NC_TILES)]
            qT = qkvpool.tile([H, T], f16)
            kT = qkvpool.tile([H, T], f16)
            vT = qkvpool.tile([H, T], f16)
            v1 = qkvpool.tile([128, T // 128, H + 1], f16)
            nc.vector.tensor_copy(v1[:, :, H : H + 1], ones)

            for lo, hi in ((0, 512), (512, 1024)):
                nc.sync.dma_start(xhi_all[:, :, lo:hi], xhi_src[:, :, lo:hi])
            nc.sync.dma_start(xhi_all[:, :, HALF:T], xhi_src[:, :, HALF:T])

            def proj_qk_subunits(tc512):
                # 3-pass split-fp16: Whi@xhi + Wlo@xhi + Whi@xlo, emitted as
                # three separately-schedulable sub-units sharing one psum
                # accumulation group
                state = {}

                def sub(pi, wgrp, xgrp):
                    if pi == 0:
                        state["ps"] = ps_proj.tile(
                            [128, 512], f32, tag="work", name="qk_ps"
                        )
                    qk_ps = state["ps"]
                    for c in range(NC_TILES):
                        nc.tensor.matmul(
                            qk_ps[:],
                            wgrp[c],
                            xgrp[c][:, tc512 : tc512 + 512],
                            start=(pi == 0 and c == 0),
                            stop=(pi == 1 and c == NC_TILES - 1),
                        )
                    if pi == 1:
                        nc.vector.tensor_copy(
                            qT[:, tc512 : tc512 + 512], qk_ps[0:H, :]
                        )
                        nc.vector.tensor_copy(
                            kT[:, tc512 : tc512 + 512], qk_ps[H : 2 * H, :]
                        )

                passes = [(wqk_hi, xhi), (wqk_lo, xhi)]
                return [
                    (lambda pi=pi, w=w, xg=xg: sub(pi, w, xg))
                    for pi, (w, xg) in enumerate(passes)
                ]

            def proj_qk_unit(tc512):
                for u in proj_qk_subunits(tc512):
                    u()

            def proj_v_unit(tc512):
                v_ps = ps_proj.tile([128, 512], f32, tag="work", name="v_ps")
                for c in range(NC_TILES):
                    nc.tensor.matmul(
                        v_ps[0:H, :],
                        wv[c],
                        xhi[c][:, tc512 : tc512 + 512],
                        start=(c == 0),
                        stop=(c == NC_TILES - 1),
                    )
                nc.vector.tensor_copy(vT[:, tc512 : tc512 + 512], v_ps[0:H, :])

            def vtrans_unit(jj_pair):
                vt_ps = ps_proj.tile([128, 2, H], f16, tag="work", name="vt_ps")
                for jl, jj in enumerate(jj_pair):
                    nc.tensor.transpose(
                        vt_ps[:, jl, :],
                        vT[:, 128 * jj : 128 * (jj + 1)],
                        ident16,
                    )
                    nc.vector.tensor_copy(v1[:, jj, 0:H], vt_ps[:, jl, :])

            def attn_S(t0, jb):
                # S^T strip matmuls for one j-block; emitted one iteration
                # ahead of its exp/PV so PV(jb-1)'s exp-wait never blocks
                # S(jb) in the PE FIFO
                i_start = max(t0, 128 * jb)
                W = t0 + HALF - i_start
                s_ps = ps_s.tile([128, HALF], f32, tag="s", name="s_ps")
                for ls, le in _chunks(0, W, 512, 0):
                    nc.tensor.matmul(
                        s_ps[:, ls:le],
                        kT[:, 128 * jb : 128 * (jb + 1)],
                        qT[:, i_start + ls : i_start + le],
                        start=True,
                        stop=True,
                    )
                return s_ps

            def attn_exp_pv(t0, n_jb, pv_ps, jb, s_ps):
                i_start = max(t0, 128 * jb)
                W = t0 + HALF - i_start
                p_sb = ppool.tile([128, HALF], f16, tag="p", name="p_sb")
                nc.scalar.activation(
                    p_sb[:, 0:W],
                    s_ps[:, 0:W],
                    mybir.ActivationFunctionType.Exp,
                    scale=1.0 / 8.0,
                )
                if 128 * jb >= t0:
                    nc.vector.tensor_mul(p_sb[:, 0:128], p_sb[:, 0:128], mask16)
                # PV accumulate: chunk by global-512 (pv bank) bounds
                for gs, ge in _chunks(i_start, t0 + HALF, 512, 0):
                    ic_last_jb = min(n_jb - 1, (ge - 1) // 128)
                    nc.tensor.matmul(
                        pv_ps[:, gs - t0 : ge - t0],
                        v1[:, jb, :],
                        p_sb[:, gs - i_start : ge - i_start],
                        start=(jb == 0),
                        stop=(jb == ic_last_jb),
                    )

            # --- phase 0: projections for half 0 ---
            for tc512 in (0, 512):
                proj_qk_unit(tc512)
                proj_v_unit(tc512)
            for pair in ((0, 1), (2, 3), (4, 5), (6, 7)):
                vtrans_unit(pair)

            # --- attention pass 0, with half-1 projection units woven into
            # the PE stream to fill its exp-wait stalls ---
            h1_units = (
                [lambda: proj_v_unit(1024)]
                + proj_qk_subunits(1024)
                + [lambda: proj_v_unit(1536)]
                + proj_qk_subunits(1536)
                + [lambda: vtrans_unit((8, 9))]
            )
            def out_chunk(pv_ps, t0, c):
                lo, hi = 512 * c, 512 * (c + 1)
                out_sb = opool.tile([H + 1, 512], f32, tag="o", name="out_sb")
                nc.vector.tensor_copy(out_sb[:], pv_ps[:, lo:hi])
                nc.sync.dma_start(out_d[:, t0 + lo : t0 + hi], out_sb[:])

            pv_ps0 = ps_pv.tile([H + 1, HALF], f32, tag="pv", name="pv_ps")
            s_cur = attn_S(0, 0)
            for jb in range(8):
                s_nxt = attn_S(0, jb + 1) if jb + 1 < 8 else None
                attn_exp_pv(0, 8, pv_ps0, jb, s_cur)
                s_cur = s_nxt
                if jb == 3:
                    out_chunk(pv_ps0, 0, 0)
                if h1_units:
                    h1_units.pop(0)()
            for u in h1_units:
                u()
            out_chunk(pv_ps0, 0, 1)

            # --- attention pass 1 ---
            pv_ps1 = ps_pv.tile([H + 1, HALF], f32, tag="pv", name="pv_ps")
            s_cur = attn_S(HALF, 0)
            for jb in range(16):
                s_nxt = attn_S(HALF, jb + 1) if jb + 1 < 16 else None
                attn_exp_pv(HALF, 16, pv_ps1, jb, s_cur)
                s_cur = s_nxt
                if jb == 11:
                    out_chunk(pv_ps1, HALF, 0)
                if jb == 8:
                    vtrans_unit((10, 11))
                elif jb == 9:
                    vtrans_unit((12, 13))
                elif jb == 10:
                    vtrans_unit((14, 15))
            out_chunk(pv_ps1, HALF, 1)

    _legalize_waits(nc)
    return nc


def build_in_maps(x, Wq, Wk, Wv):
    x = np.ascontiguousarray(np.asarray(x), dtype=np.float32)
    wqk_np = np.ascontiguousarray(
        np.concatenate([np.asarray(Wq), np.asarray(Wk)], axis=1), dtype=np.float32
    )
    wv_np = np.ascontiguousarray(np.asarray(Wv), dtype=np.float32)

    def ctile_pack(a, w):  # [512, w] -> [128, 4*w] with c-tiles side by side
        return a.reshape(4, 128, w).transpose(1, 0, 2).reshape(128, 4 * w)

    wqk_hi = wqk_np.astype(np.float16)
    wqk_lo = (wqk_np - wqk_hi.astype(np.float32)).astype(np.float16)
    mask_np = np.triu(np.ones((128, 128), dtype=np.float16))
    ident_np = np.zeros((128, H), dtype=np.float16)
    ident_np[:H] = np.eye(H, dtype=np.float16)
    ones_np = np.ones((128, T // 128), dtype=np.float16)
    consts_np = np.ascontiguousarray(
        np.concatenate(
            [
                ctile_pack(wqk_hi, 128),
                ctile_pack(wqk_lo, 128),
                ctile_pack(wv_np.astype(np.float16), 64),
                mask_np,
                ones_np,
                ident_np,
            ],
            axis=1,
        )
    )

    def reorder(a):  # [512, T] -> row (4p + c) holds row (128c + p)
        return np.ascontiguousarray(
            a.reshape(4, 128, T).transpose(1, 0, 2).reshape(D, T)
        )

    maps = []
    for b in range(N_CORES):
        xhi = x[b].T.astype(np.float16)
        maps.append({"xhi": reorder(xhi), "consts": consts_np})
    return maps


def kernel(x, Wq, Wk, Wv):
    from concourse.bass_utils import run_bass_kernel_spmd

    if "nc" not in _cache:
        _cache["nc"] = _build()
    nc = _cache["nc"]

    in_maps = build_in_maps(x, Wq, Wk, Wv)
    res = run_bass_kernel_spmd(nc, in_maps, list(range(N_CORES))).results

    out = np.empty((B, T, H), dtype=np.float32)
    for b in range(N_CORES):
        strip = res[b]["out"]  # [H+1, T]
        out[b] = (strip[:H, :] / strip[H : H + 1, :]).T
    return out


if __name__ == "__main__":
    rng = np.random.default_rng(0)
    x = rng.standard_normal((B, T, D)).astype(np.float32)
    s = 1.0 / np.sqrt(D)
    Wq = (rng.standard_normal((D, H)) * s).astype(np.float32)
    Wk = (rng.standard_normal((D, H)) * s).astype(np.float32)
    Wv = (rng.standard_normal((D, H)) * s).astype(np.float32)
    out = kernel(x=x, Wq=Wq, Wk=Wk, Wv=Wv)
    print("out", out.shape, out.dtype, np.abs(out).max())
